# revision 29
# baseline (speedup 1.0000x reference)
"""GAT (graph attention) layer on 8 Trainium2 NeuronCores.

Reference math (per head h):
    Wh = x @ W[h];  f_src = Wh @ a_src[h];  f_dst = Wh @ a_dst[h]
    e[i,j] = leaky_relu(f_src[i] + f_dst[j], alpha)
    att = softmax(where(adj>0, e, -9e15), axis=j)
    out[:, h*D:(h+1)*D] = att @ Wh

Exact identity used (lrelu(s) = max(alpha*s, s), so exp(lrelu(s)) is a
max of two separable products; the softmax row scale cancels):
    nh[j,i] = adj[j,i] * v_j * max(u_i, r_j)
      u_i = exp((1-a)*f_src_i),  v_j = exp(f_dst_j),  r_j = exp(-(1-a)*f_dst_j)
    out_h[i,:] = (sum_j nh[j,i]*Wh[j,:]) / (sum_j nh[j,i])
With q_j = r_j*v_j = exp(a*f_dst_j) a head can equivalently accumulate
    (q*Whaug).T @ adj  +  Whaug.T @ (relu(v*u - q) * adj)
because v*max(u,r) = q + relu(v*u - q).  The relu term is one ACT op
with per-partition scale (v) and bias (-q) APs, so the max moves to the
otherwise idle ACT engine and the weights stay plain.

The baseline's big per-element exp disappears: only per-vector exps
remain.  Per j-block engine assignment:
    heads 0,1 (DVE): m = (Ub max r_j)*v_j [tensor_scalar 4x]; nh = m*adj [2x]
    head 2: cols [0:SIG) relu path (ACT d2; DVE mask; PE q2-adj stream),
            cols [SIG:) DVE path
    head 3: full relu path (ACT d3; GPSIMD mask; PE q3-adj stream;
            q3-weights scaled just-in-time on GPSIMD)

Distribution: destination rows i sharded 1024/core; host passes adj.T
column-slices pre-converted to bf16 (halves DMA vs int32, no on-device
convert) and x as bf16 (feeds DMA xbar transposes, no PE transposes).
"""

import os
import numpy as np
from contextlib import ExitStack

import concourse.bass as bass
import concourse.tile as tile
from concourse import bacc, mybir
from concourse.bass_utils import run_bass_kernel_spmd
from concourse.masks import make_identity

N = 8192
DIN = 256
DOUT = 64
H = 4
NCORES = 8
SL = N // NCORES          # 1024 i's per core
NB = N // 128             # 64 j-blocks
SB = SL // 128            # 8 i-blocks per core
ALPHA = 0.2
W4C = H * DOUT            # 256
WAUG = H * (DOUT + 1)     # 260
H2, H3 = H - 2, H - 1     # relu-path heads

f32 = mybir.dt.float32
bf16 = mybir.dt.bfloat16
EXP = mybir.ActivationFunctionType.Exp
COPY = mybir.ActivationFunctionType.Copy
RELU = mybir.ActivationFunctionType.Relu
MAX = mybir.AluOpType.max
MULT = mybir.AluOpType.mult

_CACHE = {}

NBLIM = int(os.environ.get("NBLIM", str(NB)))
SIG = int(os.environ.get("SIGMA", "512"))   # head-2 relu-path column count
GRP = int(os.environ.get("GRP", "2"))       # whf blocks per psum tile
GBUF = int(os.environ.get("GBUF", "4"))     # whf psum tile buffers
RVC = 32                                    # r/v chunking in j-blocks


def _build_module():
    nc = bacc.Bacc("TRN2", target_bir_lowering=False, debug=False,
                   num_devices=NCORES)

    x_d = nc.dram_tensor("x_bf", [N, DIN], bf16, kind="ExternalInput").ap()
    xs_d = nc.dram_tensor("xs_bf", [SL, DIN], bf16, kind="ExternalInput").ap()
    w_d = nc.dram_tensor("w_all", [H, DIN, DOUT], f32, kind="ExternalInput").ap()
    a_d = nc.dram_tensor("a_all", [H, 2, DOUT], f32, kind="ExternalInput").ap()
    adjT_d = nc.dram_tensor("adjT_bf", [N, SL], bf16, kind="ExternalInput").ap()
    out_d = nc.dram_tensor("out_slice", [SL, H * DOUT], f32, kind="ExternalOutput").ap()

    with tile.TileContext(nc) as tc, ExitStack() as ctx:
        # ---------------- persistent tiles ----------------
        persist = ctx.enter_context(tc.tile_pool(name="persist", bufs=1))
        what_sb = persist.tile([128, NB, WAUG], bf16)   # plain Wh_aug, all heads
        whq2_sb = persist.tile([128, NB, DOUT + 1], bf16)  # q2-scaled head-2 wts
        xt_sb = persist.tile([128, 2, N], bf16)         # x.T via DMA xbar
        xst_sb = persist.tile([128, 2, SL], bf16)       # x_slice.T via DMA xbar
        ub_sb = persist.tile([128, H, SL], bf16)        # u_i broadcast per head
        fall_sb = persist.tile([128, NB, 2 * H], f32)   # [f_src(4) | f_dst(4)]
        r_sb = persist.tile([128, 3, NB], f32)          # exp(-(1-a)g) heads 0-2
        v_sb = persist.tile([128, H, NB], f32)          # exp(g)
        q_sb = persist.tile([128, 2, NB], f32)          # exp(a*g) h2,h3
        qn_sb = persist.tile([128, 2, NB], f32)         # -q h2,h3 (ACT bias)
        ps_sb = persist.tile([DOUT + 1, H, SL], f32)    # phase C staging
        ident_sb = persist.tile([128, 128], f32)
        make_identity(nc, ident_sb[:])

        # ======================= PHASE A =======================
        with ExitStack() as actx:
            a1pool = actx.enter_context(tc.tile_pool(name="aphase1", bufs=1))

            # denominator (aug) columns are plain ones for every head
            for h in range(H):
                nc.vector.memset(what_sb[:, :, h * (DOUT + 1) + DOUT], 1.0)

            # --- W4 and a columns ---
            w4_sb = a1pool.tile([128, 2, W4C], f32)   # [d-part, d-chunk, h*64+o]
            for h in range(H):
                nc.sync.dma_start(
                    w4_sb[:, :, h * DOUT:(h + 1) * DOUT],
                    w_d[h].rearrange("(c p) o -> p c o", p=128))
            a_bc = a1pool.tile([128, H, 2, DOUT], f32)
            nc.gpsimd.dma_start(
                a_bc[:],
                bass.AP(tensor=a_d.tensor, offset=a_d.offset,
                        ap=[[0, 128]] + list(a_d.ap)))

            # --- wtilde[d, (s h)] = sum_o W4[d, o]*a[o]  (DVE reduce) ---
            wtl_sb = a1pool.tile([128, 2, 2, H], f32)
            ttr_dump = a1pool.tile([128, DOUT], f32)
            for c in range(2):
                for s in range(2):
                    for h in range(H):
                        nc.vector.scalar_tensor_tensor(
                            out=ttr_dump[:],
                            in0=w4_sb[:, c, h * DOUT:(h + 1) * DOUT],
                            scalar=1.0,
                            in1=a_bc[:, h, s, :],
                            op0=MULT, op1=MULT,
                            accum_out=wtl_sb[:, c, s, h:h + 1])
            wf_b = a1pool.tile([128, 2, W4C + 8], bf16)  # [W4 | wtilde(src,dst)]
            nc.vector.tensor_copy(
                wf_b[:, :, W4C:],
                wtl_sb[:].rearrange("p c s h -> p c (s h)"))
            nc.vector.tensor_copy(wf_b[:, :, 0:W4C], w4_sb[:])

            # --- x_slice chain first: xsT -> f_src -> u -> Ub broadcast ---
            with ExitStack() as sctx:
                fpsum = sctx.enter_context(
                    tc.tile_pool(name="apsum_f", bufs=2, space="PSUM"))
                for c in range(2):
                    nc.sync.dma_start(
                        xst_sb[:, c, :], xs_d[:, c * 128:(c + 1) * 128],
                        transpose=True)
                # only the f projections are needed for the slice: 8-col matmuls
                fs_sb = a1pool.tile([128, SB, 4], f32)
                for bi in range(SB):
                    whf_s = fpsum.tile([128, 8], f32, tag="whfs")
                    for c in range(2):
                        nc.tensor.matmul(
                            whf_s[:], xst_sb[:, c, bi * 128:(bi + 1) * 128],
                            wf_b[:, c, W4C:], start=(c == 0), stop=(c == 1))
                    nc.vector.tensor_copy(fs_sb[:, bi, :], whf_s[:, 0:4])
                # u = exp((1-a) f_src), transpose to [32,128], broadcast via PE
                us_sb = a1pool.tile([128, SB, 4], f32)
                nc.scalar.activation(
                    out=us_sb[:].rearrange("p b h -> p (b h)"),
                    in_=fs_sb[:].rearrange("p b h -> p (b h)"),
                    func=EXP, scale=(1.0 - ALPHA))
                usT = fpsum.tile([SB * 4, 128], f32, tag="usT", bufs=1)
                nc.tensor.transpose(
                    usT[:], us_sb[:].rearrange("p b h -> p (b h)"), ident_sb[:])
                usT_sb = a1pool.tile([SB * 4, 128], bf16)
                nc.vector.tensor_copy(usT_sb[:], usT[:])
                # sel[:, bi, h, :]: [32,128], ones in row (bi*4+h)
                sel_sb = a1pool.tile([SB * 4, SB, H, 128], bf16)
                nc.gpsimd.memset(sel_sb[:], 0.0)
                nc.gpsimd.affine_select(
                    out=sel_sb[:], in_=sel_sb[:],
                    compare_op=mybir.AluOpType.not_equal,
                    fill=1.0, base=0,
                    pattern=[[4, SB], [1, H], [0, 128]],
                    channel_multiplier=-1)
                for h in range(H):
                    fbp = fpsum.tile([128, SL], f32, tag="fbp", bufs=1)
                    for bi in range(SB):
                        nc.tensor.matmul(
                            fbp[:, bi * 128:(bi + 1) * 128],
                            sel_sb[:, bi, h, :], usT_sb[:],
                            start=True, stop=True)
                    nc.vector.tensor_copy(ub_sb[:, h, :], fbp[:])

            # --- xT via DMA xbar transpose, slab by slab ---
            XSLAB = 1024
            for c in range(2):
                for s0 in range(0, N, XSLAB):
                    nc.sync.dma_start(
                        xt_sb[:, c, s0:s0 + XSLAB],
                        x_d[s0:s0 + XSLAB, c * 128:(c + 1) * 128],
                        transpose=True)

            # --- [Wh | f] per n-block, GRP blocks per psum tile ---
            with ExitStack() as sctx:
                apsum = sctx.enter_context(
                    tc.tile_pool(name="apsum_x", bufs=GBUF, space="PSUM"))
                for g0 in range(0, NB, GRP):
                    # one PSUM bank (512 f32) per block: no matmul output
                    # crosses a bank boundary
                    whf = apsum.tile([128, GRP, 512], f32, tag="whf")
                    for k in range(GRP):
                        b = g0 + k
                        for c in range(2):
                            nc.tensor.matmul(
                                whf[:, k, 0:W4C + 8],
                                xt_sb[:, c, b * 128:(b + 1) * 128],
                                wf_b[:, c, :], start=(c == 0), stop=(c == 1))
                    nc.scalar.activation(
                        out=what_sb[:, g0:g0 + GRP, :]
                            .rearrange("p g (h o) -> p g h o", h=H)[:, :, :, 0:DOUT],
                        in_=whf[:, :, 0:W4C]
                            .rearrange("p g (h o) -> p g h o", h=H),
                        func=COPY)
                    nc.vector.tensor_copy(fall_sb[:, g0:g0 + GRP, :],
                                          whf[:, :, W4C:W4C + 8])
                    # every RVC blocks: r/v/q/qn chunk + head-2 q-weights
                    if (g0 + GRP) % RVC == 0:
                        cs = slice(g0 + GRP - RVC, g0 + GRP)
                        for h in range(3):
                            nc.scalar.activation(
                                out=r_sb[:, h, cs], in_=fall_sb[:, cs, H + h],
                                func=EXP, scale=-(1.0 - ALPHA))
                        for h in range(H):
                            nc.scalar.activation(
                                out=v_sb[:, h, cs], in_=fall_sb[:, cs, H + h],
                                func=EXP, scale=1.0)
                        for e, h in enumerate((H2, H3)):
                            nc.scalar.activation(
                                out=q_sb[:, e, cs], in_=fall_sb[:, cs, H + h],
                                func=EXP, scale=ALPHA)
                            nc.vector.tensor_scalar(
                                out=qn_sb[:, e, cs], in0=q_sb[:, e, cs],
                                scalar1=-1.0, scalar2=None, op0=MULT)
                        for jb in range(cs.start, cs.stop):
                            hsl = slice(H2 * (DOUT + 1), (H2 + 1) * (DOUT + 1))
                            nc.vector.tensor_scalar(
                                out=whq2_sb[:, jb, :], in0=what_sb[:, jb, hsl],
                                scalar1=q_sb[:, 0, jb:jb + 1],
                                scalar2=None, op0=MULT)

        # ======================= PHASE B =======================
        with ExitStack() as bctx:
            bpool = bctx.enter_context(tc.tile_pool(name="bphase", bufs=4))
            bpsum = bctx.enter_context(
                tc.tile_pool(name="bpsum", bufs=1, space="PSUM"))
            ps = [bpsum.tile([DOUT + 1, SL], f32, tag=f"acc{h}", name=f"acc{h}")
                  for h in range(H)]

            # Software pipeline: the relu-path masks x2/x3 sit at the end of
            # long chains (adj -> ACT d -> mask), so their consuming matmuls
            # are emitted one block late to keep the in-order PE stream from
            # blocking on them.
            carry = None  # (jb, x2, x3) of the previous block

            def emit_relu_matmuls(cjb, x2, x3):
                wh2 = what_sb[:, cjb, H2 * (DOUT + 1):(H2 + 1) * (DOUT + 1)]
                for lo, hi in ((0, 512), (512, SIG)):
                    if hi <= lo:
                        continue
                    nc.tensor.matmul(
                        ps[H2][:, lo:hi], wh2, x2[:, lo:hi],
                        start=False, stop=(cjb == NBLIM - 1))
                wh3 = what_sb[:, cjb, H3 * (DOUT + 1):(H3 + 1) * (DOUT + 1)]
                for half in range(2):
                    nc.tensor.matmul(
                        ps[H3][:, half * 512:(half + 1) * 512], wh3,
                        x3[:, half * 512:(half + 1) * 512],
                        start=False, stop=(cjb == NBLIM - 1))

            for jb in range(NBLIM):
                adj_b = bpool.tile([128, SL], bf16, tag="adjb")
                nc.sync.dma_start(adj_b[:], adjT_d[jb * 128:(jb + 1) * 128, :])
                # ACT: d = relu(v*u - q) for relu-path heads
                d3 = bpool.tile([128, SL], bf16, tag="d3")
                nc.scalar.activation(
                    out=d3[:], in_=ub_sb[:, H3, :], func=RELU,
                    scale=v_sb[:, H3, jb:jb + 1], bias=qn_sb[:, 1, jb:jb + 1])
                d2 = bpool.tile([128, SIG], bf16, tag="d2")
                nc.scalar.activation(
                    out=d2[:], in_=ub_sb[:, H2, 0:SIG], func=RELU,
                    scale=v_sb[:, H2, jb:jb + 1], bias=qn_sb[:, 0, jb:jb + 1])
                # GPSIMD: head-3 q-weights JIT + mask
                whq3 = bpool.tile([128, DOUT + 1], bf16, tag="whq3")
                nc.gpsimd.tensor_scalar(
                    out=whq3[:],
                    in0=what_sb[:, jb, H3 * (DOUT + 1):(H3 + 1) * (DOUT + 1)],
                    scalar1=q_sb[:, 1, jb:jb + 1], scalar2=None, op0=MULT)
                x3 = bpool.tile([128, SL], bf16, tag="x3")
                nc.gpsimd.tensor_tensor(
                    out=x3[:], in0=d3[:], in1=adj_b[:], op=MULT)
                # DVE: adj-independent tensor_scalar ops first
                m0 = bpool.tile([128, SL], bf16, tag="m0")
                m1 = bpool.tile([128, SL], bf16, tag="m1")
                nc.vector.tensor_scalar(
                    out=m0[:], in0=ub_sb[:, 0, :],
                    scalar1=r_sb[:, 0, jb:jb + 1], scalar2=v_sb[:, 0, jb:jb + 1],
                    op0=MAX, op1=MULT)
                nc.vector.tensor_scalar(
                    out=m1[:], in0=ub_sb[:, 1, :],
                    scalar1=r_sb[:, 1, jb:jb + 1], scalar2=v_sb[:, 1, jb:jb + 1],
                    op0=MAX, op1=MULT)
                if SIG < SL:
                    m2 = bpool.tile([128, SL - SIG], bf16, tag="m2")
                    nc.vector.tensor_scalar(
                        out=m2[:], in0=ub_sb[:, H2, SIG:],
                        scalar1=r_sb[:, H2, jb:jb + 1],
                        scalar2=v_sb[:, H2, jb:jb + 1], op0=MAX, op1=MULT)
                # DVE masks (need adj), then x2 (needs d2) last
                nh0 = bpool.tile([128, SL], bf16, tag="nh0")
                nc.vector.tensor_tensor(out=nh0[:], in0=m0[:], in1=adj_b[:], op=MULT)
                nh1 = bpool.tile([128, SL], bf16, tag="nh1")
                nc.vector.tensor_tensor(out=nh1[:], in0=m1[:], in1=adj_b[:], op=MULT)
                if SIG < SL:
                    nh2 = bpool.tile([128, SL - SIG], bf16, tag="nh2")
                    nc.vector.tensor_tensor(
                        out=nh2[:], in0=m2[:], in1=adj_b[:, SIG:], op=MULT)
                x2 = bpool.tile([128, SIG], bf16, tag="x2")
                nc.vector.tensor_tensor(
                    out=x2[:], in0=d2[:], in1=adj_b[:, 0:SIG], op=MULT)
                # PE: previous block's relu-path streams first (ready), then
                # this block's fast streams
                if carry is not None:
                    emit_relu_matmuls(*carry)
                carry = (jb, x2, x3)
                for h, nh in ((0, nh0), (1, nh1)):
                    wh = what_sb[:, jb, h * (DOUT + 1):(h + 1) * (DOUT + 1)]
                    for half in range(2):
                        nc.tensor.matmul(
                            ps[h][:, half * 512:(half + 1) * 512], wh,
                            nh[:, half * 512:(half + 1) * 512],
                            start=(jb == 0), stop=(jb == NBLIM - 1))
                wh2a = whq2_sb[:, jb, :]
                for lo, hi in ((0, 512), (512, SIG)):
                    if hi <= lo:
                        continue
                    nc.tensor.matmul(
                        ps[H2][:, lo:hi], wh2a, adj_b[:, lo:hi],
                        start=(jb == 0), stop=False)
                if SIG < SL:
                    wh2 = what_sb[:, jb, H2 * (DOUT + 1):(H2 + 1) * (DOUT + 1)]
                    nc.tensor.matmul(
                        ps[H2][:, SIG:], wh2, nh2[:],
                        start=(jb == 0), stop=(jb == NBLIM - 1))
                for half in range(2):
                    nc.tensor.matmul(
                        ps[H3][:, half * 512:(half + 1) * 512], whq3[:],
                        adj_b[:, half * 512:(half + 1) * 512],
                        start=(jb == 0), stop=False)
            if carry is not None:
                emit_relu_matmuls(*carry)

            for h in range(H):
                nc.vector.tensor_copy(ps_sb[:, h, :], ps[h][:])

        # ======================= PHASE C =======================
        with ExitStack() as cctx:
            c2pool = cctx.enter_context(tc.tile_pool(name="c2", bufs=2))
            cpsum = cctx.enter_context(
                tc.tile_pool(name="cpsum", bufs=2, space="PSUM"))
            for bi in range(SB):
                o_sb = c2pool.tile([128, H * DOUT], f32, tag="osb")
                for h in range(H):
                    pst = cpsum.tile([128, DOUT + 1], f32, tag="pst")
                    nc.tensor.transpose(
                        pst[:], ps_sb[:, h, bi * 128:(bi + 1) * 128],
                        ident_sb[0:DOUT + 1, 0:DOUT + 1])
                    rec = c2pool.tile([128, 1], f32, tag="rec")
                    nc.vector.reciprocal(rec[:], pst[:, DOUT:DOUT + 1])
                    nc.vector.tensor_scalar_mul(
                        o_sb[:, h * DOUT:(h + 1) * DOUT], pst[:, 0:DOUT], rec[:])
                nc.sync.dma_start(out_d[bi * 128:(bi + 1) * 128, :], o_sb[:])

    nc.compile()
    return nc


def kernel(x, adj, W, a_src, a_dst):
    import ml_dtypes
    x_bf = np.ascontiguousarray(
        np.asarray(x, dtype=np.float32).astype(ml_dtypes.bfloat16))
    adj = np.asarray(adj, dtype=np.int32)
    W = np.ascontiguousarray(np.asarray(W, dtype=np.float32))
    a_all = np.ascontiguousarray(
        np.stack([np.asarray(a_src, np.float32),
                  np.asarray(a_dst, np.float32)], axis=1))  # [H, 2, DOUT]
    # adj.T as exact bf16 {0.0, 1.0}: 1.0 is 0x3F80 in bf16 bits
    adjT_bf = (adj.T.astype(np.uint16) * np.uint16(0x3F80)).view(
        ml_dtypes.bfloat16)
    adjT_bf = np.ascontiguousarray(adjT_bf)

    if "nc" not in _CACHE:
        _CACHE["nc"] = _build_module()
    nc = _CACHE["nc"]

    in_maps = []
    for c in range(NCORES):
        sl = slice(c * SL, (c + 1) * SL)
        in_maps.append({
            "x_bf": x_bf,
            "xs_bf": np.ascontiguousarray(x_bf[sl]),
            "w_all": W,
            "a_all": a_all,
            "adjT_bf": np.ascontiguousarray(adjT_bf[:, sl]),
        })
    res = run_bass_kernel_spmd(nc, in_maps, core_ids=list(range(NCORES)))
    out = np.concatenate([res.results[c]["out_slice"] for c in range(NCORES)],
                         axis=0)
    return out
